# revision 14
# baseline (speedup 1.0000x reference)
"""Angular prototypical loss on 8 TRN2 NeuronCores (Bass/Tile, SPMD).

kernel(**inputs): takes FULL inputs (embeddings [65536,256] f32, labels
[65536] i32, num_classes), shards the batch across the 8 cores, runs one
SPMD Bass kernel (split AllReduce of per-class prototype sums on-chip),
returns the scalar mean loss.

Per-core algorithm (rows = 8192 = 64 tiles of 128):
  Phase A: cast x->bf16; nsq via fused mul-reduce; invn = exp(-0.5 ln nsq)
    (stays in the exp/ln ACT table set); one-hot scaled by invn via one
    dual-op tensor_scalar; S^T += x^T @ oh via 4 N=512 matmuls/tile
    (2 LDWEIGHTS).  Batch split in two halves -> two AllReduces, the
    first overlapping the second half of Phase A.
  Transition: S^T -> (xbar transpose) class-major S; per-class norm via
    fused mul-reduce + exp/ln rsqrt; scaled prototypes written to DRAM
    (gather source) and transposed back to d-major for Phase B.
  Phase B: cos row-tile via 4 N=512 matmuls (d-chunk outer, 2 LDWEIGHTS);
    exp with per-partition scale=invn/T AP reads PSUM directly, row-sum
    via the ACT accumulator; m = x . shat[label] via one fused
    tensor_tensor_reduce on the indirect-DMA-gathered prototype rows.
  Epilogue: batched [128, 64] margin/CE math, sin via exp(0.5 ln(1-m^2)).
"""
import numpy as np
from concourse.bass_utils import run_bass_kernel_spmd

import math

import concourse.bass as bass
import concourse.mybir as mybir
import concourse.tile as tile
import concourse.bacc as bacc

P = 128
D = 256
C = 1024
NCORES = 8
MARGIN = 0.2
INV_T = 10.0
COS_M = math.cos(MARGIN)
SIN_M = math.sin(MARGIN)
TH = math.cos(math.pi - MARGIN)

f32 = mybir.dt.float32
bf16 = mybir.dt.bfloat16
fp16 = mybir.dt.float16
i32 = mybir.dt.int32

AF = mybir.ActivationFunctionType
OP = mybir.AluOpType


def build(nt: int = 64, group: int = 8, debug_taps: bool = False,
          split_ar: bool = True, use_scale_ap: bool = True,
          dot_mode: str = "amr", use_gather: bool = True):
    """nt: row-tiles per core (rows/core = 128*nt). group: tiles per DMA group."""
    BL = P * nt
    ng = nt // group
    half = nt // 2
    assert nt % group == 0 and nt % 2 == 0

    nc = bacc.Bacc("TRN2", target_bir_lowering=False, debug=False,
                   num_devices=NCORES)
    emb = nc.declare_dram_parameter("embeddings", [BL, D], f32, isOutput=False)
    lab = nc.declare_dram_parameter("labels", [BL], i32, isOutput=False)
    out = nc.declare_dram_parameter("out", [P, 1], f32, isOutput=True)
    if debug_taps:
        dbg_sg = nc.declare_dram_parameter("dbg_sg", [P, 2 * C], f32,
                                           isOutput=True)
        dbg_sc = nc.declare_dram_parameter("dbg_sc", [P, 16 * P], f32,
                                           isOutput=True)
        dbg_pnsq = nc.declare_dram_parameter("dbg_pnsq", [P, 8], f32,
                                             isOutput=True)
        dbg_nsq = nc.declare_dram_parameter("dbg_nsq", [P, nt], f32,
                                            isOutput=True)
        dbg_m = nc.declare_dram_parameter("dbg_m", [P, nt], f32,
                                          isOutput=True)
        dbg_se = nc.declare_dram_parameter("dbg_se", [P, nt], f32,
                                           isOutput=True)
        dbg_h1 = nc.declare_dram_parameter("dbg_h1", [P, 2 * C], f32,
                                           isOutput=True)
        dbg_a1 = nc.declare_dram_parameter("dbg_a1", [P, 2 * C], f32,
                                           isOutput=True)
        dbg_a2 = nc.declare_dram_parameter("dbg_a2", [P, 2 * C], f32,
                                           isOutput=True)

    emb_g = emb.ap().rearrange("(p q) d -> p q d", p=P)      # [128, nt, 256]
    lab_pn = lab.ap().rearrange("(p n) -> p n", p=P)         # [128, nt]

    with tile.TileContext(nc) as tc:
        with (
            tc.tile_pool(name="big", bufs=1) as big,
            tc.tile_pool(name="stage", bufs=2) as stage,
            tc.tile_pool(name="ohp", bufs=4) as ohp,
            tc.tile_pool(name="gat", bufs=4) as gat,
            tc.tile_pool(name="scr", bufs=2) as scr,
            tc.tile_pool(name="expp", bufs=2) as expp,
            tc.tile_pool(name="dram", bufs=1, space="DRAM") as dram,
        ):
            ar_in0 = dram.tile([P, 2 * C], bf16, tag="ar_in0")
            ar_in1 = dram.tile([P, 2 * C], bf16, tag="ar_in1")
            ar_out0 = dram.tile([P, 2 * C], bf16, tag="ar_out0",
                                addr_space="Shared")
            ar_out1 = dram.tile([P, 2 * C], bf16, tag="ar_out1",
                                addr_space="Shared")
            shat_dram = dram.tile([C, D], bf16, tag="shat_dram")
            # [c, d] with c = 128*j + p, d = 128*k + r  ->  [p, k, j, r]
            shat_dram_v = shat_dram.rearrange("(j p) (k r) -> p k j r",
                                              p=P, k=2)

            # ---- persistent SBUF ----
            x_bf = big.tile([P, nt * D], bf16, tag="x_bf")
            xT = big.tile([P, nt, 2, P], bf16, tag="xT")
            sT = big.tile([P, 2, 8, P], bf16, tag="sT")      # [d%128, dk, j, c%128]
            sC = big.tile([P, 2, 8, P], bf16, tag="sC")      # [c%128, dk, j, d%128]
            sg = big.tile([P, 2, C], bf16, tag="sg")
            ar1_sb = big.tile([P, 2 * C], bf16, tag="ar1_sb")
            ar2_sb = big.tile([P, 2 * C], bf16, tag="ar2_sb")
            s_h1 = big.tile([P, 2, C], bf16, tag="s_h1")
            s_h2 = big.tile([P, 2, C], bf16, tag="s_h2")
            lab_i = big.tile([P, nt], i32, tag="lab_i")
            lab_f = big.tile([P, nt], f32, tag="lab_f")
            nsq = big.tile([P, nt], f32, tag="nsq")
            lnt = big.tile([P, nt], f32, tag="lnt")
            invn = big.tile([P, nt], f32, tag="invn")
            invnT = big.tile([P, nt], f32, tag="invnT")
            m_raw = big.tile([P, nt], f32, tag="m_raw")
            sumexp = big.tile([P, nt], f32, tag="sumexp")
            pnsq = big.tile([P, 8], f32, tag="pnsq")
            plnt = big.tile([P, 8], f32, tag="plnt")
            pinv = big.tile([P, 8], f32, tag="pinv")
            iota16 = big.tile([P, C], fp16, tag="iota16")

            nc.gpsimd.iota(iota16[:], pattern=[[1, C]], base=0,
                           channel_multiplier=0,
                           allow_small_or_imprecise_dtypes=True)
            nc.sync.dma_start(out=lab_i[:], in_=lab_pn)
            nc.vector.tensor_copy(lab_f[:], lab_i[:])

            # ================= Phase A =================
            with tc.tile_pool(name="psA", bufs=1, space="PSUM") as psA:
                # 8 accumulators: [half][dchunk][chalf], each one PSUM bank
                sacc = [[[psA.tile([P, 512], f32, tag=f"sacc{h}{dk}{ch}",
                                   name=f"sacc{h}{dk}{ch}")
                          for ch in range(2)] for dk in range(2)]
                        for h in range(2)]
                for g in range(ng):
                    raw = stage.tile([P, group, D], f32, tag="raw")
                    nc.sync.dma_start(out=raw[:],
                                      in_=emb_g[:, g * group:(g + 1) * group, :])
                    gsl = slice(g * group, (g + 1) * group)
                    for t in range(group):
                        n = g * group + t
                        x_n = x_bf[:, n * D:(n + 1) * D]
                        nc.vector.tensor_copy(x_n, raw[:, t, :])
                        tr = scr.tile([P, D], bf16, tag="tr")
                        if dot_mode == "ttr":
                            nc.vector.tensor_tensor_reduce(
                                out=tr[:], in0=x_n, in1=x_n, scale=1.0,
                                scalar=0.0, op0=OP.mult, op1=OP.add,
                                accum_out=nsq[:, n:n + 1])
                        elif dot_mode == "amr":
                            nc.vector.affine_mul_reduce(
                                out=tr[:], accum_out=nsq[:, n:n + 1],
                                in0=x_n, in1=x_n, scale=1.0, bias=0.0)
                        else:
                            nc.vector.tensor_tensor(tr[:], x_n, x_n,
                                                    op=OP.mult)
                            nc.vector.reduce_sum(nsq[:, n:n + 1], tr[:],
                                                 axis=mybir.AxisListType.X)
                    # invn = nsq^(-1/2) via exp/ln (no sqrt table switch)
                    nc.scalar.activation(lnt[:, gsl], nsq[:, gsl], AF.Ln)
                    nc.scalar.activation(invn[:, gsl], lnt[:, gsl], AF.Exp,
                                         scale=-0.5)
                    for t in range(group):
                        n = g * group + t
                        x_n = x_bf[:, n * D:(n + 1) * D]
                        oh = ohp.tile([P, C], bf16, tag="oh")
                        nc.vector.tensor_scalar(
                            oh[:], iota16[:], lab_f[:, n:n + 1],
                            invn[:, n:n + 1], OP.is_equal, OP.mult)
                        h = n // half
                        st = (n % half == 0)
                        sp = (n % half == half - 1)
                        for dk in range(2):
                            lhsT = x_bf[:, n * D + dk * P:n * D + (dk + 1) * P]
                            for ch in range(2):
                                nc.tensor.matmul(
                                    out=sacc[h][dk][ch][:],
                                    lhsT=lhsT,
                                    rhs=oh[:, ch * 512:(ch + 1) * 512],
                                    start=st, stop=sp)
                    nc.sync.dma_start_transpose(
                        out=xT[:, g * group:(g + 1) * group, :, :],
                        in_=x_bf[:, g * group * D:(g + 1) * group * D])
                    if split_ar and (g + 1) * group == half:
                        # first-half proto sums -> DRAM -> AllReduce #1
                        for dk in range(2):
                            for ch in range(2):
                                nc.scalar.copy(
                                    s_h1[:, dk, ch * 512:(ch + 1) * 512],
                                    sacc[0][dk][ch][:])
                        nc.sync.dma_start(out=ar_in0[:], in_=s_h1[:])
                        nc.gpsimd.collective_compute(
                            "AllReduce", OP.add,
                            replica_groups=[list(range(NCORES))],
                            ins=[ar_in0[:].opt()], outs=[ar_out0[:].opt()])
                if not split_ar:
                    for dk in range(2):
                        for ch in range(2):
                            nc.scalar.copy(
                                s_h1[:, dk, ch * 512:(ch + 1) * 512],
                                sacc[0][dk][ch][:])
                # second half -> AllReduce #2
                for dk in range(2):
                    for ch in range(2):
                        nc.scalar.copy(s_h2[:, dk, ch * 512:(ch + 1) * 512],
                                       sacc[1][dk][ch][:])
                if split_ar:
                    nc.sync.dma_start(out=ar_in1[:], in_=s_h2[:])
                    nc.gpsimd.collective_compute(
                        "AllReduce", OP.add,
                        replica_groups=[list(range(NCORES))],
                        ins=[ar_in1[:].opt()], outs=[ar_out1[:].opt()])

            nc.vector.tensor_scalar_mul(invnT[:], invn[:], INV_T)

            if split_ar:
                nc.sync.dma_start(out=ar1_sb[:], in_=ar_out0[:])
                nc.sync.dma_start(out=ar2_sb[:], in_=ar_out1[:])
                nc.vector.tensor_tensor(sg[:].rearrange("p k c -> p (k c)"),
                                        ar1_sb[:], ar2_sb[:], op=OP.add)
            else:
                nc.vector.tensor_tensor(
                    s_h1[:].rearrange("p k c -> p (k c)"),
                    s_h1[:].rearrange("p k c -> p (k c)"),
                    s_h2[:].rearrange("p k c -> p (k c)"), op=OP.add)
                nc.sync.dma_start(out=ar_in0[:], in_=s_h1[:])
                nc.gpsimd.collective_compute(
                    "AllReduce", OP.add,
                    replica_groups=[list(range(NCORES))],
                    ins=[ar_in0[:].opt()], outs=[ar_out0[:].opt()])
                nc.sync.dma_start(
                    out=sg[:].rearrange("p k c -> p (k c)"), in_=ar_out0[:])

            # ---- S^T -> class-major, normalize, -> DRAM + back ----
            # sC[p, k, j, r] = S[c = 128j + p, d = 128k + r]
            nc.sync.dma_start_transpose(
                out=sC[:], in_=sg[:].rearrange("p k c -> p (k c)"))
            for j in range(8):
                trj = scr.tile([P, 2, P], bf16, tag="trj")
                if dot_mode == "ttr":
                    nc.vector.tensor_tensor_reduce(
                        out=trj[:], in0=sC[:, :, j, :], in1=sC[:, :, j, :],
                        scale=1.0, scalar=0.0, op0=OP.mult, op1=OP.add,
                        accum_out=pnsq[:, j:j + 1])
                elif dot_mode == "amr":
                    nc.vector.affine_mul_reduce(
                        out=trj[:], accum_out=pnsq[:, j:j + 1],
                        in0=sC[:, :, j, :], in1=sC[:, :, j, :],
                        scale=1.0, bias=0.0)
                else:
                    nc.vector.tensor_tensor(trj[:], sC[:, :, j, :],
                                            sC[:, :, j, :], op=OP.mult)
                    nc.vector.reduce_sum(pnsq[:, j:j + 1], trj[:],
                                         axis=mybir.AxisListType.XY)
            if debug_taps:
                dsg = big.tile([P, 2 * C], f32, tag="dsg")
                nc.vector.tensor_copy(dsg[:], sg[:].rearrange("p k c -> p (k c)"))
                nc.sync.dma_start(out=dbg_sg.ap(), in_=dsg[:])
                dh1 = big.tile([P, 2 * C], f32, tag="dh1")
                nc.vector.tensor_copy(dh1[:], s_h1[:].rearrange("p k c -> p (k c)"))
                nc.sync.dma_start(out=dbg_h1.ap(), in_=dh1[:])
                da1 = big.tile([P, 2 * C], f32, tag="da1")
                nc.vector.tensor_copy(da1[:], ar1_sb[:])
                nc.sync.dma_start(out=dbg_a1.ap(), in_=da1[:])
                da2 = big.tile([P, 2 * C], f32, tag="da2")
                nc.vector.tensor_copy(da2[:], ar2_sb[:])
                nc.sync.dma_start(out=dbg_a2.ap(), in_=da2[:])
                dsc = big.tile([P, 16 * P], f32, tag="dsc")
                nc.vector.tensor_copy(dsc[:], sC[:].rearrange("p k j r -> p (k j r)"))
                nc.sync.dma_start(out=dbg_sc.ap(), in_=dsc[:])
                nc.sync.dma_start(out=dbg_pnsq.ap(), in_=pnsq[:])
                nc.sync.dma_start(out=dbg_nsq.ap(), in_=nsq[:])
            nc.scalar.activation(plnt[:], pnsq[:], AF.Ln)
            nc.scalar.activation(pinv[:], plnt[:], AF.Exp, scale=-0.5)
            for j in range(8):
                nc.vector.tensor_scalar_mul(sC[:, :, j, :], sC[:, :, j, :],
                                            pinv[:, j:j + 1])
            nc.sync.dma_start(out=shat_dram_v, in_=sC[:])
            # sT[q, k, j, p] = shat[c = 128j + p, d = 128k + q]
            nc.sync.dma_start_transpose(
                out=sT[:], in_=sC[:].rearrange("p k j r -> p (k j r)"))

            # ================= Phase B =================
            with tc.tile_pool(name="psB", bufs=2, space="PSUM") as psB:
                for n in range(nt):
                    Gt = gat.tile([P, D], bf16, tag=f"G{n % 4}",
                                  name=f"G_{n}")
                    if use_gather:
                        nc.gpsimd.indirect_dma_start(
                            out=Gt[:], out_offset=None,
                            in_=shat_dram[:],
                            in_offset=bass.IndirectOffsetOnAxis(
                                ap=lab_i[:, n:n + 1], axis=0))
                    else:
                        nc.vector.tensor_copy(Gt[:], x_bf[:, n * D:(n + 1) * D])
                    cos_ps = psB.tile([P, C], f32, tag="cos")
                    for dk in range(2):
                        for ch in range(2):
                            nc.tensor.matmul(
                                out=cos_ps[:, ch * 512:(ch + 1) * 512],
                                lhsT=xT[:, n, dk, :],
                                rhs=sT[:, dk, ch * 4:(ch + 1) * 4, :],
                                start=(dk == 0), stop=(dk == 1))
                    exps = expp.tile([P, C], bf16, tag="exps")
                    nc.scalar.activation(
                        exps[:], cos_ps[:], AF.Exp,
                        scale=(invnT[:, n:n + 1] if use_scale_ap else INV_T),
                        accum_out=sumexp[:, n:n + 1])
                    trm = scr.tile([P, D], bf16, tag="trm")
                    if dot_mode == "ttr":
                        nc.vector.tensor_tensor_reduce(
                            out=trm[:], in0=x_bf[:, n * D:(n + 1) * D],
                            in1=Gt[:], scale=1.0, scalar=0.0,
                            op0=OP.mult, op1=OP.add,
                            accum_out=m_raw[:, n:n + 1])
                    elif dot_mode == "amr":
                        nc.vector.affine_mul_reduce(
                            out=trm[:], accum_out=m_raw[:, n:n + 1],
                            in0=x_bf[:, n * D:(n + 1) * D], in1=Gt[:],
                            scale=1.0, bias=0.0)
                    else:
                        nc.vector.tensor_tensor(
                            trm[:], x_bf[:, n * D:(n + 1) * D], Gt[:],
                            op=OP.mult)
                        nc.vector.reduce_sum(m_raw[:, n:n + 1], trm[:],
                                             axis=mybir.AxisListType.X)

            if debug_taps:
                nc.sync.dma_start(out=dbg_m.ap(), in_=m_raw[:])
                nc.sync.dma_start(out=dbg_se.ap(), in_=sumexp[:])

            # ================= epilogue (batched [P, nt]) ================
            m_all = big.tile([P, nt], f32, tag="m_all")
            b1 = big.tile([P, nt], f32, tag="b1")
            b2 = big.tile([P, nt], f32, tag="b2")
            b3 = big.tile([P, nt], f32, tag="b3")
            b4 = big.tile([P, nt], f32, tag="b4")
            mask = big.tile([P, nt], mybir.dt.uint8, tag="mask")
            phi_f = big.tile([P, nt], f32, tag="phi_f")

            nc.vector.tensor_tensor(m_all[:], m_raw[:], invn[:], op=OP.mult)
            nc.vector.tensor_tensor(b1[:], m_all[:], m_all[:], op=OP.mult)
            nc.vector.tensor_scalar(b1[:], b1[:], -1.0, 1.0, OP.mult, OP.add)
            nc.vector.tensor_scalar_max(b1[:], b1[:], 0.0)
            # sin = sqrt(b1) = exp(0.5 ln b1)  (ln 0 -> -inf -> exp -> 0)
            nc.scalar.activation(b2[:], b1[:], AF.Ln)
            nc.scalar.activation(b2[:], b2[:], AF.Exp, scale=0.5)
            nc.vector.tensor_scalar_mul(b3[:], m_all[:], COS_M)
            nc.vector.tensor_scalar(b2[:], b2[:], -SIN_M, None, OP.mult)
            nc.vector.tensor_add(b3[:], b3[:], b2[:])           # phi
            nc.vector.tensor_scalar(mask[:], m_all[:], TH, None, OP.is_gt)
            nc.vector.tensor_scalar(b4[:], m_all[:], -MARGIN, None, OP.add)
            nc.vector.select(phi_f[:], mask[:], b3[:], b4[:])
            nc.scalar.activation(b1[:], m_all[:], AF.Exp, scale=INV_T)
            nc.scalar.activation(b2[:], phi_f[:], AF.Exp, scale=INV_T)
            nc.vector.tensor_sub(b1[:], sumexp[:], b1[:])
            nc.vector.tensor_add(b1[:], b1[:], b2[:])           # Z
            nc.scalar.activation(b2[:], b1[:], AF.Ln)
            nc.vector.tensor_scalar_mul(b3[:], phi_f[:], INV_T)
            nc.vector.tensor_sub(b2[:], b2[:], b3[:])           # nll
            part = big.tile([P, 1], f32, tag="part")
            nc.vector.reduce_sum(part[:], b2[:], axis=mybir.AxisListType.X)
            nc.sync.dma_start(out=out[:], in_=part[:])

    nc.compile()
    return nc


_NC_CACHE = {}


def kernel(embeddings, labels, num_classes=None, **_ignored):
    embeddings = np.ascontiguousarray(embeddings, dtype=np.float32)
    labels = np.ascontiguousarray(labels, dtype=np.int32)
    B = embeddings.shape[0]
    BL = B // NCORES

    if "nc" not in _NC_CACHE:
        _NC_CACHE["nc"] = build()
    nc = _NC_CACHE["nc"]

    in_maps = [{"embeddings": embeddings[i * BL:(i + 1) * BL],
                "labels": labels[i * BL:(i + 1) * BL]}
               for i in range(NCORES)]
    res = run_bass_kernel_spmd(nc, in_maps, list(range(NCORES)))
    total = 0.0
    for i in range(NCORES):
        total += res.results[i]["out"].astype(np.float64).sum()
    return np.float32(total / B)


# revision 17
# speedup vs baseline: 1.0292x; 1.0292x over previous
"""Angular prototypical loss on 8 TRN2 NeuronCores (Bass/Tile, SPMD).

kernel(**inputs): takes FULL inputs (embeddings [65536,256] f32, labels
[65536] i32, num_classes), shards the batch across the 8 cores, runs one
SPMD Bass kernel (AllReduce of per-class prototype sums on-chip), returns
the scalar mean loss.

Per-core algorithm (rows = 8192 = 64 tiles of 128):
  Phase A: cast x->bf16 (DVE); row norm^2 on ScalarE (Square + ACT
    accumulator); invn = exp(-0.5 ln nsq) (stays in the exp/ln ACT table
    set, no sqrt table switch); one-hot scaled by invn via one dual-op
    tensor_scalar; S^T += x^T @ oh via 4 N=512 matmuls/tile (2 LDWEIGHTS,
    d-chunk outer).
  Transition: AllReduce S^T (bf16); xbar-transpose to class-major;
    per-class norm via fused affine_mul_reduce + exp/ln rsqrt; transpose
    back to d-major for Phase B.
  Phase B: cos row-tile via 4 N=512 matmuls (d-chunk outer, 2 LDWEIGHTS);
    exp with per-partition scale=invn/T AP reads PSUM directly, row-sum
    via the ACT accumulator; exp(m/T) extracted from the exps tile by a
    per-partition [label, label+1) tensor_mask_reduce (dot_mode="mask"),
    or m = x . shat[label] via indirect-DMA gather + affine_mul_reduce
    (dot_mode="amr").
  Epilogue: batched [128, 64] margin/CE math, sqrt via exp(0.5 ln x).
"""
import numpy as np
from concourse.bass_utils import run_bass_kernel_spmd

import math

import concourse.bass as bass
import concourse.mybir as mybir
import concourse.tile as tile
import concourse.bacc as bacc

P = 128
D = 256
C = 1024
NCORES = 8
MARGIN = 0.2
INV_T = 10.0
T = 0.1
COS_M = math.cos(MARGIN)
SIN_M = math.sin(MARGIN)
TH = math.cos(math.pi - MARGIN)

f32 = mybir.dt.float32
bf16 = mybir.dt.bfloat16
fp16 = mybir.dt.float16
i32 = mybir.dt.int32

AF = mybir.ActivationFunctionType
OP = mybir.AluOpType


def build(nt: int = 64, group: int = 8, dot_mode: str = "amr"):
    """nt: row-tiles per core (rows/core = 128*nt). group: tiles per DMA group."""
    BL = P * nt
    ng = nt // group
    assert nt % group == 0

    nc = bacc.Bacc("TRN2", target_bir_lowering=False, debug=False,
                   num_devices=NCORES)
    emb = nc.declare_dram_parameter("embeddings", [BL, D], f32, isOutput=False)
    lab = nc.declare_dram_parameter("labels", [BL], i32, isOutput=False)
    out = nc.declare_dram_parameter("out", [P, 1], f32, isOutput=True)

    emb_g = emb.ap().rearrange("(p q) d -> p q d", p=P)      # [128, nt, 256]
    lab_pn = lab.ap().rearrange("(p n) -> p n", p=P)         # [128, nt]

    with tile.TileContext(nc) as tc:
        with (
            tc.tile_pool(name="big", bufs=1) as big,
            tc.tile_pool(name="stage", bufs=2) as stage,
            tc.tile_pool(name="ohp", bufs=4) as ohp,
            tc.tile_pool(name="gat", bufs=4) as gat,
            tc.tile_pool(name="scr", bufs=2) as scr,
            tc.tile_pool(name="expp", bufs=2) as expp,
            tc.tile_pool(name="dram", bufs=1, space="DRAM") as dram,
        ):
            ar_in = dram.tile([P, 2 * C], bf16, tag="ar_in")
            ar_out = dram.tile([P, 2 * C], bf16, tag="ar_out",
                               addr_space="Shared")
            if dot_mode == "amr":
                shat_dram = dram.tile([C, D], bf16, tag="shat_dram")
                shat_dram_v = shat_dram.rearrange("(j p) (k r) -> p k j r",
                                                  p=P, k=2)

            # ---- persistent SBUF ----
            x_bf = big.tile([P, nt * D], bf16, tag="x_bf")
            xT = big.tile([P, nt, 2, P], bf16, tag="xT")
            sT = big.tile([P, 2, 8, P], bf16, tag="sT")   # [d%128, dk, j, c%128]
            sC = big.tile([P, 2, 8, P], bf16, tag="sC")   # [c%128, dk, j, d%128]
            sg = big.tile([P, 2, C], bf16, tag="sg")
            s_loc = big.tile([P, 2, C], bf16, tag="s_loc")
            lab_i = big.tile([P, nt], i32, tag="lab_i")
            lab_f = big.tile([P, nt], f32, tag="lab_f")
            labp1 = big.tile([P, nt], f32, tag="labp1")
            nsq = big.tile([P, nt], f32, tag="nsq")
            lnt = big.tile([P, nt], f32, tag="lnt")
            invn = big.tile([P, nt], f32, tag="invn")
            m_raw = big.tile([P, nt], f32, tag="m_raw")
            sumexp = big.tile([P, nt], f32, tag="sumexp")
            pnsq = big.tile([P, 8], f32, tag="pnsq")
            plnt = big.tile([P, 8], f32, tag="plnt")
            pinv = big.tile([P, 8], f32, tag="pinv")
            iota16 = big.tile([P, C], fp16, tag="iota16")

            nc.gpsimd.iota(iota16[:], pattern=[[1, C]], base=0,
                           channel_multiplier=0,
                           allow_small_or_imprecise_dtypes=True)
            nc.sync.dma_start(out=lab_i[:], in_=lab_pn)
            nc.vector.tensor_copy(lab_f[:], lab_i[:])
            nc.vector.tensor_scalar(labp1[:], lab_f[:], 1.0, None, OP.add)

            # ================= Phase A =================
            with tc.tile_pool(name="psA", bufs=1, space="PSUM") as psA:
                sacc = [[psA.tile([P, 512], f32, tag=f"sacc{dk}{ch}",
                                  name=f"sacc{dk}{ch}")
                         for ch in range(2)] for dk in range(2)]
                for g in range(ng):
                    raw = stage.tile([P, group, D], f32, tag="raw")
                    nc.sync.dma_start(out=raw[:],
                                      in_=emb_g[:, g * group:(g + 1) * group, :])
                    gsl = slice(g * group, (g + 1) * group)
                    for t in range(group):
                        n = g * group + t
                        x_n = x_bf[:, n * D:(n + 1) * D]
                        nc.vector.tensor_copy(x_n, raw[:, t, :])
                        sq = scr.tile([P, D], f32, tag="sq")
                        nc.scalar.activation(sq[:], raw[:, t, :], AF.Square,
                                             accum_out=nsq[:, n:n + 1])
                    # invn = nsq^(-1/2) via exp/ln (no sqrt table switch)
                    nc.scalar.activation(lnt[:, gsl], nsq[:, gsl], AF.Ln)
                    nc.scalar.activation(invn[:, gsl], lnt[:, gsl], AF.Exp,
                                         scale=-0.5)
                    for t in range(group):
                        n = g * group + t
                        x_n = x_bf[:, n * D:(n + 1) * D]
                        nc.vector.tensor_scalar_mul(x_n, x_n,
                                                    invn[:, n:n + 1])
                        oh = ohp.tile([P, C], bf16, tag="oh")
                        nc.vector.tensor_scalar(
                            oh[:], iota16[:], lab_f[:, n:n + 1],
                            None, OP.is_equal)
                        for dk in range(2):
                            lhsT = x_bf[:, n * D + dk * P:n * D + (dk + 1) * P]
                            for ch in range(2):
                                nc.tensor.matmul(
                                    out=sacc[dk][ch][:],
                                    lhsT=lhsT,
                                    rhs=oh[:, ch * 512:(ch + 1) * 512],
                                    start=(n == 0), stop=(n == nt - 1))
                    nc.sync.dma_start_transpose(
                        out=xT[:, g * group:(g + 1) * group, :, :],
                        in_=x_bf[:, g * group * D:(g + 1) * group * D])
                # proto sums -> DRAM -> AllReduce
                for dk in range(2):
                    for ch in range(2):
                        nc.scalar.copy(s_loc[:, dk, ch * 512:(ch + 1) * 512],
                                       sacc[dk][ch][:])
            nc.sync.dma_start(out=ar_in[:], in_=s_loc[:])
            nc.gpsimd.collective_compute(
                "AllReduce", OP.add,
                replica_groups=[list(range(NCORES))],
                ins=[ar_in[:].opt()], outs=[ar_out[:].opt()])
            nc.sync.dma_start(out=sg[:].rearrange("p k c -> p (k c)"),
                              in_=ar_out[:])

            # ---- S^T -> class-major, normalize, back to d-major ----
            # sC[p, k, j, r] = S[c = 128j + p, d = 128k + r]
            nc.sync.dma_start_transpose(
                out=sC[:], in_=sg[:].rearrange("p k c -> p (k c)"))
            for j in range(8):
                trj = scr.tile([P, 2, P], bf16, tag="trj")
                nc.vector.affine_mul_reduce(
                    out=trj[:], accum_out=pnsq[:, j:j + 1],
                    in0=sC[:, :, j, :], in1=sC[:, :, j, :],
                    scale=1.0, bias=0.0)
            nc.scalar.activation(plnt[:], pnsq[:], AF.Ln)
            nc.scalar.activation(pinv[:], plnt[:], AF.Exp, scale=-0.5)
            for j in range(8):
                nc.vector.tensor_scalar_mul(sC[:, :, j, :], sC[:, :, j, :],
                                            pinv[:, j:j + 1])
            if dot_mode == "amr":
                nc.sync.dma_start(out=shat_dram_v, in_=sC[:])
            # sT[q, k, j, p] = shat[c = 128j + p, d = 128k + q]
            nc.sync.dma_start_transpose(
                out=sT[:], in_=sC[:].rearrange("p k j r -> p (k j r)"))

            # ================= Phase B =================
            with tc.tile_pool(name="psB", bufs=2, space="PSUM") as psB:
                for n in range(nt):
                    if dot_mode == "amr":
                        Gt = gat.tile([P, D], bf16, tag=f"G{n % 4}",
                                      name=f"G_{n}")
                        nc.gpsimd.indirect_dma_start(
                            out=Gt[:], out_offset=None,
                            in_=shat_dram[:],
                            in_offset=bass.IndirectOffsetOnAxis(
                                ap=lab_i[:, n:n + 1], axis=0))
                    cos_ps = psB.tile([P, C], f32, tag="cos")
                    for dk in range(2):
                        for ch in range(2):
                            nc.tensor.matmul(
                                out=cos_ps[:, ch * 512:(ch + 1) * 512],
                                lhsT=xT[:, n, dk, :],
                                rhs=sT[:, dk, ch * 4:(ch + 1) * 4, :],
                                start=(dk == 0), stop=(dk == 1))
                    exps = expp.tile([P, C], bf16, tag="exps")
                    nc.scalar.activation(
                        exps[:], cos_ps[:], AF.Exp, scale=INV_T,
                        accum_out=sumexp[:, n:n + 1])
                    if dot_mode == "amr":
                        trm = scr.tile([P, D], bf16, tag="trm")
                        nc.vector.affine_mul_reduce(
                            out=trm[:], accum_out=m_raw[:, n:n + 1],
                            in0=x_bf[:, n * D:(n + 1) * D], in1=Gt[:],
                            scale=1.0, bias=0.0)
                    else:
                        # m_raw[p] = exp(m/T) = exps[p, label[p]]
                        trm = scr.tile([P, C], bf16, tag="trm")
                        nc.vector.tensor_mask_reduce(
                            out=trm[:], in_=exps[:],
                            mask_start=lab_f[:, n:n + 1],
                            mask_end=labp1[:, n:n + 1],
                            scale=1.0, accum_in=0.0, op=OP.max,
                            accum_out=m_raw[:, n:n + 1])

            # ================= epilogue (batched [P, nt]) ================
            m_all = big.tile([P, nt], f32, tag="m_all")
            expm = big.tile([P, nt], f32, tag="expm")
            b1 = big.tile([P, nt], f32, tag="b1")
            b2 = big.tile([P, nt], f32, tag="b2")
            b3 = big.tile([P, nt], f32, tag="b3")
            b4 = big.tile([P, nt], f32, tag="b4")
            mask = big.tile([P, nt], mybir.dt.uint8, tag="mask")
            phi_f = big.tile([P, nt], f32, tag="phi_f")

            if dot_mode == "amr":
                nc.vector.tensor_copy(m_all[:], m_raw[:])
                nc.scalar.activation(expm[:], m_all[:], AF.Exp, scale=INV_T)
            else:
                # m = T ln(exp(m/T));  expm = exp(m/T) directly
                nc.vector.tensor_copy(expm[:], m_raw[:])
                nc.scalar.activation(b1[:], m_raw[:], AF.Ln)
                nc.vector.tensor_scalar_mul(m_all[:], b1[:], T)
            nc.vector.tensor_tensor(b1[:], m_all[:], m_all[:], op=OP.mult)
            nc.vector.tensor_scalar(b1[:], b1[:], -1.0, 1.0, OP.mult, OP.add)
            nc.vector.tensor_scalar_max(b1[:], b1[:], 0.0)
            # sin = sqrt(b1) = exp(0.5 ln b1)  (ln 0 -> -inf -> exp -> 0)
            nc.scalar.activation(b2[:], b1[:], AF.Ln)
            nc.scalar.activation(b2[:], b2[:], AF.Exp, scale=0.5)
            nc.vector.tensor_scalar_mul(b3[:], m_all[:], COS_M)
            nc.vector.tensor_scalar(b2[:], b2[:], -SIN_M, None, OP.mult)
            nc.vector.tensor_add(b3[:], b3[:], b2[:])           # phi
            nc.vector.tensor_scalar(mask[:], m_all[:], TH, None, OP.is_gt)
            nc.vector.tensor_scalar(b4[:], m_all[:], -MARGIN, None, OP.add)
            nc.vector.select(phi_f[:], mask[:], b3[:], b4[:])
            nc.scalar.activation(b2[:], phi_f[:], AF.Exp, scale=INV_T)
            nc.vector.tensor_sub(b1[:], sumexp[:], expm[:])
            nc.vector.tensor_add(b1[:], b1[:], b2[:])           # Z'
            nc.scalar.activation(b2[:], b1[:], AF.Ln)
            nc.vector.tensor_scalar_mul(b3[:], phi_f[:], INV_T)
            nc.vector.tensor_sub(b2[:], b2[:], b3[:])           # nll
            part = big.tile([P, 1], f32, tag="part")
            nc.vector.reduce_sum(part[:], b2[:], axis=mybir.AxisListType.X)
            nc.sync.dma_start(out=out[:], in_=part[:])

    nc.compile()
    return nc


_NC_CACHE = {}


def kernel(embeddings, labels, num_classes=None, **_ignored):
    embeddings = np.ascontiguousarray(embeddings, dtype=np.float32)
    labels = np.ascontiguousarray(labels, dtype=np.int32)
    B = embeddings.shape[0]
    BL = B // NCORES

    if "nc" not in _NC_CACHE:
        _NC_CACHE["nc"] = build()
    nc = _NC_CACHE["nc"]

    in_maps = [{"embeddings": embeddings[i * BL:(i + 1) * BL],
                "labels": labels[i * BL:(i + 1) * BL]}
               for i in range(NCORES)]
    res = run_bass_kernel_spmd(nc, in_maps, list(range(NCORES)))
    total = 0.0
    for i in range(NCORES):
        total += res.results[i]["out"].astype(np.float64).sum()
    return np.float32(total / B)
